# revision 1
# baseline (speedup 1.0000x reference)
"""PointPillarScatter on 8 TRN2 cores via PE one-hot matmul.

Scatter -> dense-matmul transform: host packs pillars (sorted by output
column) into 32-slot windows per 128-column tile.  On device, a one-hot
matrix P[slot, col] = (colof[slot] == col) is built with a single Vector
is_equal per 2 tiles (iota constant vs per-slot column offset, empty slots
get -1 so their row is all-zero), then PSUM[col, feat] = P^T @ feats gives
every output element exactly once (fp32 matmul of a 0/1 matrix is exact).

HW constraints found empirically: matmul operands at base partition 64
fault the exec unit (only 0/32 safe), and multiple accumulation groups
per PSUM bank fault.  So tiles rotate over 2 partition blocks {0,32} and
every matmul owns a full PSUM bank (out at bank offset 0).

Sharding: core k owns flat output columns [k*88000, (k+1)*88000) of the
5*140800 (cav, y, x) space; 688 tiles of 128 cols per core.  8 matmuls
(8 banks) per chunk are Act-copied into one SBUF stage tile [128, 512],
then one 256KB DMA out.  Host re-assembles [5, 64, 200, 704].
"""

import numpy as np

import concourse.bass as bass
import concourse.tile as tile
from concourse import mybir
from concourse.bass_utils import run_bass_kernel_spmd

NUM_FEATURES = 64
MAX_CAV = 5
NX, NY = 704, 200
NUM_PIXELS = NY * NX            # 140800
TOTAL = MAX_CAV * NUM_PIXELS    # 704000
N_CORES = 8
CORE_COLS = TOTAL // N_CORES    # 88000 flat columns per core
TILE_COLS = 128
N_TILES = 688                   # 688*128 = 88064 >= 88000
SLOTS = 32                      # max pillars per tile (seed-0 max is 23)
BLKS = N_TILES // 2             # 344: 2 tiles share one is_equal
CHUNKS = N_TILES // 8           # 86: 8 tiles per out-DMA chunk
OUT_W = N_TILES * NUM_FEATURES  # 44032

_PROG = None


def _split_excess_waits(nc, max_waits=1):
    """Walrus enforces tight per-instruction sync-wait encoding limits. Spill
    surplus waits onto single-wait EventSemaphore nops inserted just before
    the offending instruction on the same engine queue (same semantics:
    engine blocks at the nop, then proceeds)."""
    for blk in nc.main_func.blocks:
        i = 0
        while i < len(blk.instructions):
            inst = blk.instructions[i]
            si = inst.sync_info
            if si is None or len(si.on_wait) <= max_waits:
                i += 1
                continue
            waits = list(si.on_wait)
            keep, spill = waits[-max_waits:], waits[:-max_waits]
            for w in spill:
                nop = mybir.InstEventSemaphore(
                    name=f"I-{nc.next_id()}", ins=[], outs=[]
                )
                nop.engine = inst.engine
                nop.sync_info = mybir.SyncInfo(on_wait=[w], on_update=[])
                nc.register_instruction(nop)
                blk.instructions.insert(i, nop)
                i += 1
            si.on_wait = keep
            inst.sync_info = si
            i += 1


def _build_prog():
    f32 = mybir.dt.float32
    nc = bass.Bass()
    # feats: tile t = 2*b+k lives at partitions [32k, 32k+32), free [64b, 64b+64)
    feats = nc.dram_tensor("feats", [64, BLKS * 64], f32, kind="ExternalInput")
    colof = nc.dram_tensor("colof", [64, BLKS], f32, kind="ExternalInput")
    iota = nc.dram_tensor("iota", [64, 128], f32, kind="ExternalInput")
    # out[p, t*64+f] = feature f of tile t's column p
    out = nc.dram_tensor("out", [128, OUT_W], f32, kind="ExternalOutput")

    with tile.TileContext(nc) as tc:
        with (
            tc.tile_pool(name="const", bufs=1) as constp,
            tc.tile_pool(name="pmat", bufs=3) as pmatp,
            tc.tile_pool(name="psum", bufs=8, space="PSUM") as psump,
            tc.tile_pool(name="stage", bufs=3) as stagep,
        ):
            feats_sb = constp.tile([64, BLKS * 64], f32)
            nc.sync.dma_start(feats_sb[:], feats[:])
            colof_sb = constp.tile([64, BLKS], f32)
            nc.sync.dma_start(colof_sb[:], colof[:])
            iota_sb = constp.tile([64, 128], f32)
            nc.sync.dma_start(iota_sb[:], iota[:])

            P = None
            cur_b = -1
            for c in range(CHUNKS):
                st = stagep.tile([128, 512], f32)
                for j in range(8):
                    t = 8 * c + j
                    b, k = t // 2, t % 2
                    if b != cur_b:
                        P = pmatp.tile([64, 128], f32)
                        nc.vector.tensor_tensor(
                            out=P[:],
                            in0=colof_sb[:, b:b + 1].to_broadcast([64, 128]),
                            in1=iota_sb[:],
                            op=mybir.AluOpType.is_equal,
                        )
                        cur_b = b
                    ps = psump.tile([128, 512], f32, space="PSUM")
                    nc.tensor.matmul(
                        out=ps[:, 0:64],
                        lhsT=P[32 * k:32 * (k + 1), :],
                        rhs=feats_sb[32 * k:32 * (k + 1), b * 64:(b + 1) * 64],
                        start=True,
                        stop=True,
                    )
                    nc.scalar.activation(
                        st[:, j * 64:(j + 1) * 64],
                        ps[:, 0:64],
                        mybir.ActivationFunctionType.Copy,
                    )
                nc.sync.dma_start(out[:, c * 512:(c + 1) * 512], st[:])
    _split_excess_waits(nc)
    return nc


def _host_prep(voxel_coords, pillar_features):
    vc = voxel_coords.astype(np.int64)
    flat = vc[:, 0] * NUM_PIXELS + vc[:, 2] * NX + vc[:, 3]
    feats = np.ascontiguousarray(pillar_features, dtype=np.float32)
    core = flat // CORE_COLS
    rem = flat - core * CORE_COLS
    t = rem // TILE_COLS
    cof = rem - t * TILE_COLS
    k = t % 2
    blk = t // 2
    # slot = rank of pillar within its (core, tile) group
    order = np.argsort(flat, kind="stable")
    gid_sorted = (core * N_TILES + t)[order]
    rank_sorted = np.arange(len(flat)) - np.searchsorted(
        gid_sorted, gid_sorted, side="left"
    )
    slot = np.empty(len(flat), np.int64)
    slot[order] = rank_sorted
    assert slot.max() < SLOTS, f"tile overflow: {slot.max() + 1} slots"
    row = k * SLOTS + slot

    iota_arr = np.broadcast_to(
        np.arange(128, dtype=np.float32), (64, 128)
    ).copy()
    in_maps = []
    for cidx in range(N_CORES):
        m = core == cidx
        fa = np.zeros((64, BLKS, 64), np.float32)
        ca = np.full((64, BLKS), -1.0, np.float32)
        ca[row[m], blk[m]] = cof[m]
        fa[row[m], blk[m], :] = feats[m]
        in_maps.append({
            "feats": fa.reshape(64, BLKS * 64),
            "colof": ca,
            "iota": iota_arr,
        })
    return in_maps


def _unshard(core_outs):
    full = np.empty((TOTAL, NUM_FEATURES), np.float32)
    for cidx, o in enumerate(core_outs):       # o: [128, OUT_W]
        r = o.reshape(128, N_TILES, 64).transpose(1, 0, 2)
        r = r.reshape(N_TILES * 128, 64)
        full[cidx * CORE_COLS:(cidx + 1) * CORE_COLS] = r[:CORE_COLS]
    return np.ascontiguousarray(
        full.reshape(MAX_CAV, NUM_PIXELS, NUM_FEATURES)
        .transpose(0, 2, 1)
        .reshape(MAX_CAV, NUM_FEATURES, NY, NX)
    )


def kernel(voxel_coords, pillar_features):
    global _PROG
    if _PROG is None:
        _PROG = _build_prog()
    in_maps = _host_prep(voxel_coords, pillar_features)
    res = run_bass_kernel_spmd(_PROG, in_maps, list(range(N_CORES)))
    return _unshard([r["out"] for r in res.results])



# revision 3
# speedup vs baseline: 2.2094x; 2.2094x over previous
"""PointPillarScatter on 8 TRN2 cores via PE one-hot matmul, fp16.

Scatter -> dense-matmul transform, one matmul per chunk of 8 canvas
tiles (tile = 128 consecutive canvas columns):

  out[64h+f, 512c + n] = sum_k F[k, 64h+f] * P[k, 512c + n]

  lhsT = F [K=128, M=128]   stationary, block-diag: slot rows [0,64)
         (pool for the 4 "half A" tiles) carry feats in cols [0,64);
         rows [64,128) (half B pool) in cols [64,128).
  rhs  = P [K=128, N=512]   one-hot, built on DVE by a single is_equal:
         P[k, n] = (colof[k] == n), colof = 128*jj + cc encodes both
         the tile-within-half jj and the column cc; empty slots -1.
  out  = PSUM [128, 512] f32, one full bank per matmul (one
         accumulation group per bank -- multi-group faults on TRN2).

Slots are pooled per (chunk, half): 4 tiles share 64 slots, so tile
occupancy only matters in aggregate (mean 43.6, cap 64).  The rare
overflowing group (1 in 1376 for seed 0) is fixed by swapping its
heaviest tile with the lightest tile of the lightest group; the
resulting tile permutation is applied on the host during unshard.

fp16 end-to-end: matmul of a 0/1 one-hot against fp16 feats is exact
in f32 PSUM, so the only error is the f32->fp16 input cast (~5e-4
relative), well under the 2e-2 gate, and memory traffic halves.

PSUM -> SBUF copies are split between Act and DVE; 8 chunks stage
into [128, 4096] fp16 then leave in one 1 MiB DMA.
"""

import numpy as np

import concourse.bass as bass
import concourse.tile as tile
from concourse import mybir
from concourse.bass_utils import run_bass_kernel_spmd

NUM_FEATURES = 64
MAX_CAV = 5
NX, NY = 704, 200
NUM_PIXELS = NY * NX            # 140800
TOTAL = MAX_CAV * NUM_PIXELS    # 704000
N_CORES = 8
CORE_COLS = TOTAL // N_CORES    # 88000 flat columns per core
TILE_COLS = 128
N_TILES = 688                   # 688*128 = 88064 >= 88000
CHUNKS = N_TILES // 8           # 86 chunks of 8 tiles
POOL = 64                       # slots shared by the 4 tiles of one half
OUT_W = CHUNKS * 512            # 44032
FEED_SPLIT = 4                  # feats strip arrives in 4 DMAs

_PROG = None


def _split_excess_waits(nc, max_waits=1):
    """Walrus enforces tight per-instruction sync-wait encoding limits. Spill
    surplus waits onto single-wait EventSemaphore nops inserted just before
    the offending instruction on the same engine queue (same semantics:
    engine blocks at the nop, then proceeds)."""
    for blk in nc.main_func.blocks:
        i = 0
        while i < len(blk.instructions):
            inst = blk.instructions[i]
            si = inst.sync_info
            if si is None or len(si.on_wait) <= max_waits:
                i += 1
                continue
            waits = list(si.on_wait)
            keep, spill = waits[-max_waits:], waits[:-max_waits]
            for w in spill:
                nop = mybir.InstEventSemaphore(
                    name=f"I-{nc.next_id()}", ins=[], outs=[]
                )
                nop.engine = inst.engine
                nop.sync_info = mybir.SyncInfo(on_wait=[w], on_update=[])
                nc.register_instruction(nop)
                blk.instructions.insert(i, nop)
                i += 1
            si.on_wait = keep
            inst.sync_info = si
            i += 1


def _build_prog():
    f16 = mybir.dt.float16
    f32 = mybir.dt.float32
    nc = bass.Bass()
    feats = nc.dram_tensor("feats", [128, CHUNKS * 128], f16, kind="ExternalInput")
    colof = nc.dram_tensor("colof", [128, CHUNKS], f16, kind="ExternalInput")
    iota = nc.dram_tensor("iota", [128, 512], f16, kind="ExternalInput")
    out = nc.dram_tensor("out", [128, OUT_W], f16, kind="ExternalOutput")

    with tile.TileContext(nc) as tc:
        with (
            tc.tile_pool(name="const", bufs=1) as constp,
            tc.tile_pool(name="pmat", bufs=3) as pmatp,
            tc.tile_pool(name="psum", bufs=8, space="PSUM") as psump,
            tc.tile_pool(name="stage", bufs=3) as stagep,
        ):
            colof_sb = constp.tile([128, CHUNKS], f16)
            nc.sync.dma_start(colof_sb[:], colof[:])
            iota_sb = constp.tile([128, 512], f16)
            nc.sync.dma_start(iota_sb[:], iota[:])
            feats_sb = constp.tile([128, CHUNKS * 128], f16)
            step = CHUNKS * 128 // FEED_SPLIT
            for i in range(FEED_SPLIT):
                nc.sync.dma_start(
                    feats_sb[:, i * step:(i + 1) * step],
                    feats[:, i * step:(i + 1) * step],
                )

            for c0 in range(0, CHUNKS, 8):
                n8 = min(8, CHUNKS - c0)
                st = stagep.tile([128, 512 * n8], f16)
                for j in range(n8):
                    c = c0 + j
                    P = pmatp.tile([128, 512], f16)
                    nc.vector.tensor_tensor(
                        out=P[:],
                        in0=colof_sb[:, c:c + 1].to_broadcast([128, 512]),
                        in1=iota_sb[:],
                        op=mybir.AluOpType.is_equal,
                    )
                    ps = psump.tile([128, 512], f32, space="PSUM")
                    nc.tensor.matmul(
                        out=ps[:],
                        lhsT=feats_sb[:, c * 128:(c + 1) * 128],
                        rhs=P[:],
                        start=True,
                        stop=True,
                    )
                    if j % 4 == 3:
                        nc.vector.tensor_scalar_add(
                            st[:, j * 512:(j + 1) * 512], ps[:], 0.0
                        )
                    else:
                        nc.scalar.activation(
                            st[:, j * 512:(j + 1) * 512],
                            ps[:],
                            mybir.ActivationFunctionType.Copy,
                        )
                nc.sync.dma_start(
                    out[:, c0 * 512:(c0 + n8) * 512], st[:]
                )
    _split_excess_waits(nc)
    return nc


def _host_prep(voxel_coords, pillar_features):
    vc = voxel_coords.astype(np.int64)
    flat = vc[:, 0] * NUM_PIXELS + vc[:, 2] * NX + vc[:, 3]
    feats = pillar_features.astype(np.float16)
    core = flat // CORE_COLS
    rem = flat - core * CORE_COLS
    t = rem // TILE_COLS            # tile within core, 0..687
    cc = rem - t * TILE_COLS        # column within tile

    iota_arr = np.broadcast_to(
        np.arange(512, dtype=np.float16), (128, 512)
    ).copy()

    in_maps = []
    perms = []
    for cidx in range(N_CORES):
        m = core == cidx
        tc_, cc_, fe_ = t[m], cc[m], feats[m]
        cnt = np.bincount(tc_, minlength=N_TILES)

        # perm[q] = original tile occupying virtual slot q; virtual slot q
        # belongs to chunk q//8, half (q%8)//4, jj q%4.
        perm = np.arange(N_TILES)
        gsum = cnt.reshape(N_TILES // 4, 4).sum(axis=1)
        for _ in range(64):
            gbad = int(np.argmax(gsum))
            if gsum[gbad] <= POOL:
                break
            glight = int(np.argmin(gsum))
            bt = gbad * 4 + int(np.argmax(cnt[perm[gbad * 4:gbad * 4 + 4]]))
            lt = glight * 4 + int(
                np.argmin(cnt[perm[glight * 4:glight * 4 + 4]])
            )
            perm[bt], perm[lt] = perm[lt], perm[bt]
            gsum[gbad] = cnt[perm[gbad * 4:gbad * 4 + 4]].sum()
            gsum[glight] = cnt[perm[glight * 4:glight * 4 + 4]].sum()
        assert gsum.max() <= POOL, f"group overflow: {gsum.max()}"
        perms.append(perm)

        pos = np.empty(N_TILES, np.int64)
        pos[perm] = np.arange(N_TILES)
        q = pos[tc_]                       # virtual tile slot per pillar
        chunk = q // 8
        h = (q % 8) // 4
        jj = q % 4
        grp = chunk * 2 + h                # slot pool id, 0..171

        # slot = rank of pillar within its pool
        order = np.argsort(grp, kind="stable")
        gs = grp[order]
        rank = np.arange(len(gs)) - np.searchsorted(gs, gs, side="left")
        slot = np.empty(len(gs), np.int64)
        slot[order] = rank
        assert slot.max() < POOL

        k = h * POOL + slot
        fa = np.zeros((128, CHUNKS, 2, 64), np.float16)
        fa[k, chunk, h, :] = fe_
        ca = np.full((128, CHUNKS), -1.0, np.float16)
        ca[k, chunk] = (jj * TILE_COLS + cc_).astype(np.float16)
        in_maps.append({
            "feats": fa.reshape(128, CHUNKS * 128),
            "colof": ca,
            "iota": iota_arr,
        })
    return in_maps, perms


def _unshard(core_outs, perms):
    full = np.empty((TOTAL, NUM_FEATURES), np.float32)
    for cidx, o in enumerate(core_outs):       # o: [128, OUT_W] fp16
        v = o.reshape(2, 64, CHUNKS, 4, 128)   # [h, f, chunk, jj, cc]
        v = v.transpose(2, 0, 3, 4, 1)         # [chunk, h, jj, cc, f]
        vt = v.reshape(N_TILES, TILE_COLS, NUM_FEATURES)
        ct = np.empty_like(vt)
        ct[perms[cidx]] = vt
        r = ct.reshape(N_TILES * TILE_COLS, NUM_FEATURES)[:CORE_COLS]
        full[cidx * CORE_COLS:(cidx + 1) * CORE_COLS] = r.astype(np.float32)
    return np.ascontiguousarray(
        full.reshape(MAX_CAV, NUM_PIXELS, NUM_FEATURES)
        .transpose(0, 2, 1)
        .reshape(MAX_CAV, NUM_FEATURES, NY, NX)
    )


def kernel(voxel_coords, pillar_features):
    global _PROG
    if _PROG is None:
        _PROG = _build_prog()
    in_maps, perms = _host_prep(voxel_coords, pillar_features)
    res = run_bass_kernel_spmd(_PROG, in_maps, list(range(N_CORES)))
    return _unshard([r["out"] for r in res.results], perms)


# revision 8
# speedup vs baseline: 3.1777x; 1.4382x over previous
"""PointPillarScatter on 8 TRN2 cores via PE one-hot matmul, fp16.

Scatter -> dense-matmul transform, one matmul per chunk of 8 canvas
tiles (tile = 128 consecutive canvas columns):

  out[64h+f, 512c + n] = sum_k F[k, 64h+f] * P[k, 512c + n]

  lhsT = F [K=128, M=128]   stationary, block-diag: slot rows [0,64)
         (pool for the 4 "half A" tiles) carry feats in cols [0,64);
         rows [64,128) (half B pool) in cols [64,128).
  rhs  = P [K=128, N=512]   one-hot, built on DVE by a single is_equal:
         P[k, n] = (colof[k] == n), colof = 128*jj + cc encodes both
         the tile-within-half jj and the column cc; empty slots -1.
  out  = PSUM [128, 512] f32, one full bank per matmul (one
         accumulation group per bank -- multi-group faults on TRN2).

Slots are pooled per (chunk, half): 4 tiles share 64 slots, so tile
occupancy only matters in aggregate (mean 43.6, cap 64).  The rare
overflowing group (1 in 1376 for seed 0) is fixed by swapping its
heaviest tile with the lightest tile of the lightest group; the
resulting tile permutation is applied on the host during unshard.

fp16 end-to-end: matmul of a 0/1 one-hot against fp16 feats is exact
in f32 PSUM, so the only error is the f32->fp16 input cast (~5e-4
relative), well under the 2e-2 gate, and memory traffic halves.

PSUM -> SBUF copies are split between Act and DVE; 8 chunks stage
into [128, 4096] fp16 then leave in one 1 MiB DMA.
"""

import numpy as np

import concourse.bass as bass
import concourse.tile as tile
from concourse import mybir
from concourse.bass_utils import run_bass_kernel_spmd

NUM_FEATURES = 64
MAX_CAV = 5
NX, NY = 704, 200
NUM_PIXELS = NY * NX            # 140800
TOTAL = MAX_CAV * NUM_PIXELS    # 704000
N_CORES = 8
CORE_COLS = TOTAL // N_CORES    # 88000 flat columns per core
TILE_COLS = 128
N_TILES = 688                   # 688*128 = 88064 >= 88000
CHUNKS = N_TILES // 8           # 86 chunks of 8 tiles
POOL = 64                       # slots shared by the 4 tiles of one half
OUT_W = CHUNKS * 512            # 44032
FEED_SPLIT = 4                  # feats strip arrives in 4 DMAs

_PROG = None


def _split_excess_waits(nc, max_waits=1):
    """Walrus enforces tight per-instruction sync-wait encoding limits. Spill
    surplus waits onto single-wait EventSemaphore nops inserted just before
    the offending instruction on the same engine queue (same semantics:
    engine blocks at the nop, then proceeds)."""
    for blk in nc.main_func.blocks:
        i = 0
        while i < len(blk.instructions):
            inst = blk.instructions[i]
            si = inst.sync_info
            if si is None or len(si.on_wait) <= max_waits:
                i += 1
                continue
            waits = list(si.on_wait)
            keep, spill = waits[-max_waits:], waits[:-max_waits]
            for w in spill:
                nop = mybir.InstEventSemaphore(
                    name=f"I-{nc.next_id()}", ins=[], outs=[]
                )
                nop.engine = inst.engine
                nop.sync_info = mybir.SyncInfo(on_wait=[w], on_update=[])
                nc.register_instruction(nop)
                blk.instructions.insert(i, nop)
                i += 1
            si.on_wait = keep
            inst.sync_info = si
            i += 1


def _build_prog():
    f16 = mybir.dt.float16
    f32 = mybir.dt.float32
    nc = bass.Bass()
    feats = nc.dram_tensor("feats", [128, CHUNKS * 128], f16, kind="ExternalInput")
    colof = nc.dram_tensor("colof", [128, CHUNKS], f32, kind="ExternalInput")
    iota = nc.dram_tensor("iota", [128, 512], f16, kind="ExternalInput")
    out = nc.dram_tensor("out", [128, OUT_W], f16, kind="ExternalOutput")

    with tile.TileContext(nc) as tc:
        with (
            tc.tile_pool(name="const", bufs=1) as constp,
            tc.tile_pool(name="pmat", bufs=3) as pmatp,
            tc.tile_pool(name="psum", bufs=4, space="PSUM") as psump,
            tc.tile_pool(name="stage", bufs=3) as stagep,
        ):
            colof_sb = constp.tile([128, CHUNKS], f32)
            nc.sync.dma_start(colof_sb[:], colof[:])
            iota_sb = constp.tile([128, 512], f16)
            nc.sync.dma_start(iota_sb[:], iota[:])
            feats_sb = constp.tile([128, CHUNKS * 128], f16)
            step = CHUNKS * 128 // FEED_SPLIT
            for i in range(FEED_SPLIT):
                nc.sync.dma_start(
                    feats_sb[:, i * step:(i + 1) * step],
                    feats[:, i * step:(i + 1) * step],
                )

            for c0 in range(0, CHUNKS, 8):
                n8 = min(8, CHUNKS - c0)
                st = stagep.tile([128, 512 * n8], f16)
                for p0 in range(0, n8, 2):
                    np_ = min(2, n8 - p0)
                    ps = psump.tile([128, 512 * np_], f32, space="PSUM")
                    for j in range(p0, p0 + np_):
                        c = c0 + j
                        P = pmatp.tile([128, 512], f16)
                        nc.vector.tensor_scalar(
                            out=P[:],
                            in0=iota_sb[:],
                            scalar1=colof_sb[:, c:c + 1],
                            scalar2=None,
                            op0=mybir.AluOpType.is_equal,
                        )
                        nc.tensor.matmul(
                            out=ps[:, (j - p0) * 512:(j - p0 + 1) * 512],
                            lhsT=feats_sb[:, c * 128:(c + 1) * 128],
                            rhs=P[:],
                            start=True,
                            stop=True,
                        )
                    # copy both banks in one instruction; split Act/DVE 3:1
                    dst = st[:, p0 * 512:(p0 + np_) * 512]
                    if (p0 // 2) % 4 == 3:
                        nc.vector.tensor_scalar_add(dst, ps[:], 0.0)
                    else:
                        nc.scalar.activation(
                            dst, ps[:], mybir.ActivationFunctionType.Copy
                        )
                nc.sync.dma_start(
                    out[:, c0 * 512:(c0 + n8) * 512], st[:]
                )
    _split_excess_waits(nc)
    return nc


def _host_prep(voxel_coords, pillar_features):
    vc = voxel_coords.astype(np.int64)
    flat = vc[:, 0] * NUM_PIXELS + vc[:, 2] * NX + vc[:, 3]
    feats = pillar_features.astype(np.float16)
    core = flat // CORE_COLS
    rem = flat - core * CORE_COLS
    t = rem // TILE_COLS            # tile within core, 0..687
    cc = rem - t * TILE_COLS        # column within tile

    iota_arr = np.broadcast_to(
        np.arange(512, dtype=np.float16), (128, 512)
    ).copy()

    in_maps = []
    perms = []
    for cidx in range(N_CORES):
        m = core == cidx
        tc_, cc_, fe_ = t[m], cc[m], feats[m]
        cnt = np.bincount(tc_, minlength=N_TILES)

        # perm[q] = original tile occupying virtual slot q; virtual slot q
        # belongs to chunk q//8, half (q%8)//4, jj q%4.
        perm = np.arange(N_TILES)
        gsum = cnt.reshape(N_TILES // 4, 4).sum(axis=1)
        for _ in range(64):
            gbad = int(np.argmax(gsum))
            if gsum[gbad] <= POOL:
                break
            glight = int(np.argmin(gsum))
            bt = gbad * 4 + int(np.argmax(cnt[perm[gbad * 4:gbad * 4 + 4]]))
            lt = glight * 4 + int(
                np.argmin(cnt[perm[glight * 4:glight * 4 + 4]])
            )
            perm[bt], perm[lt] = perm[lt], perm[bt]
            gsum[gbad] = cnt[perm[gbad * 4:gbad * 4 + 4]].sum()
            gsum[glight] = cnt[perm[glight * 4:glight * 4 + 4]].sum()
        assert gsum.max() <= POOL, f"group overflow: {gsum.max()}"
        perms.append(perm)

        pos = np.empty(N_TILES, np.int64)
        pos[perm] = np.arange(N_TILES)
        q = pos[tc_]                       # virtual tile slot per pillar
        chunk = q // 8
        h = (q % 8) // 4
        jj = q % 4
        grp = chunk * 2 + h                # slot pool id, 0..171

        # slot = rank of pillar within its pool
        order = np.argsort(grp, kind="stable")
        gs = grp[order]
        rank = np.arange(len(gs)) - np.searchsorted(gs, gs, side="left")
        slot = np.empty(len(gs), np.int64)
        slot[order] = rank
        assert slot.max() < POOL

        k = h * POOL + slot
        fa = np.zeros((128, CHUNKS, 2, 64), np.float16)
        fa[k, chunk, h, :] = fe_
        ca = np.full((128, CHUNKS), -1.0, np.float32)
        ca[k, chunk] = (jj * TILE_COLS + cc_).astype(np.float32)
        in_maps.append({
            "feats": fa.reshape(128, CHUNKS * 128),
            "colof": ca,
            "iota": iota_arr,
        })
    return in_maps, perms


def _unshard(core_outs, perms):
    full = np.empty((TOTAL, NUM_FEATURES), np.float32)
    for cidx, o in enumerate(core_outs):       # o: [128, OUT_W] fp16
        v = o.reshape(2, 64, CHUNKS, 4, 128)   # [h, f, chunk, jj, cc]
        v = v.transpose(2, 0, 3, 4, 1)         # [chunk, h, jj, cc, f]
        vt = v.reshape(N_TILES, TILE_COLS, NUM_FEATURES)
        ct = np.empty_like(vt)
        ct[perms[cidx]] = vt
        r = ct.reshape(N_TILES * TILE_COLS, NUM_FEATURES)[:CORE_COLS]
        full[cidx * CORE_COLS:(cidx + 1) * CORE_COLS] = r.astype(np.float32)
    return np.ascontiguousarray(
        full.reshape(MAX_CAV, NUM_PIXELS, NUM_FEATURES)
        .transpose(0, 2, 1)
        .reshape(MAX_CAV, NUM_FEATURES, NY, NX)
    )


def kernel(voxel_coords, pillar_features):
    global _PROG
    if _PROG is None:
        _PROG = _build_prog()
    in_maps, perms = _host_prep(voxel_coords, pillar_features)
    res = run_bass_kernel_spmd(_PROG, in_maps, list(range(N_CORES)))
    return _unshard([r["out"] for r in res.results], perms)
